# revision 14
# baseline (speedup 1.0000x reference)
"""Multi-head attention (B=2, S=2048, D=1024, H=16) on 8 Trainium2 NeuronCores.

Sharding: tensor-parallel on heads (4 groups of 4 heads) x data-parallel on
batch (2) -> 8 cores. Each core computes QKV projections for its head slice,
attention for its 4 heads, and a partial output projection; the host sums the
4 partials per batch element (the tensor-parallel allreduce) and adds bo.

v2 schedule: the PE streams matmul rows at 2.4 GHz, so the kernel floor is
~164us of moving rows plus the ACT engine's ~144us of exp. To keep both
engines saturated, projections are decomposed into small "filler" quanta
(2-4 matmuls) that are interleaved between the score/attnV matmuls of the
attention loop: K(s-half1) fills head0's first 8 k-chunks, V chunks 8-15
fill head0's last 8, Q(s-half1) fills head1, and the q-block0 output
projection fills q-block1's attention. Weights are host-packed so every
weight DMA is one contiguous copy; PSUM is split 4+2+1+1 banks.
"""

import numpy as np
from collections import deque

import concourse.bass as bass  # noqa: F401
import concourse.tile as tile
from concourse import bacc, mybir
from concourse.bass_utils import run_bass_kernel_spmd

D_MODEL = 1024
NUM_HEADS = 16
DK = 64
B, S = 2, 2048
N_CORES = 8
GROUPS = 4                 # head groups (tensor parallel)
GW = D_MODEL // GROUPS     # 256 features per group = 4 heads
HPG = GROUPS               # heads per group = 4

F32 = mybir.dt.float32
BF16 = mybir.dt.float16  # 16-bit matmul operand dtype
EXPF = mybir.ActivationFunctionType.Exp
MULT = mybir.AluOpType.mult
ADD = mybir.AluOpType.add


_UID = [0]


def _uid():
    _UID[0] += 1
    return _UID[0]


def _emit(nc, tc, ctx):
    P = 128
    xqT = nc.dram_tensor("xqT", [D_MODEL, S], BF16, kind="ExternalInput")
    xkT = nc.dram_tensor("xkT", [D_MODEL, S], BF16, kind="ExternalInput")
    xvT = nc.dram_tensor("xvT", [D_MODEL, S], BF16, kind="ExternalInput")
    wqP = nc.dram_tensor("wqP", [P, 8, GW], BF16, kind="ExternalInput")
    wkP = nc.dram_tensor("wkP", [P, 8, GW], BF16, kind="ExternalInput")
    wvP = nc.dram_tensor("wvP", [P, 8, GW], BF16, kind="ExternalInput")
    woP = nc.dram_tensor("woP", [P, 2, D_MODEL], BF16, kind="ExternalInput")
    bq2 = nc.dram_tensor("bq2", [P, 2], F32, kind="ExternalInput")
    bk2 = nc.dram_tensor("bk2", [P, 2], F32, kind="ExternalInput")
    bvr = nc.dram_tensor("bvr", [1, GW], F32, kind="ExternalInput")
    out = nc.dram_tensor("out", [S, D_MODEL], F32, kind="ExternalOutput")

    consts = ctx.enter_context(tc.tile_pool(name="consts", bufs=1))
    persist = ctx.enter_context(tc.tile_pool(name="persist", bufs=1))
    xb = ctx.enter_context(tc.tile_pool(name="xb", bufs=3))
    xs = ctx.enter_context(tc.tile_pool(name="xs", bufs=4))
    sx = ctx.enter_context(tc.tile_pool(name="stexp", bufs=2))
    nrm = ctx.enter_context(tc.tile_pool(name="nrm", bufs=3))
    outp = ctx.enter_context(tc.tile_pool(name="outp", bufs=4))
    # PSUM: psA 2x[128,1024] (4 banks) scores + pre-phase proj pairs;
    # psB 2x[128,512] (2 banks) attnV accumulators (pinned per head);
    # psD/psP 1x[128,512] each: V-proj / filler-proj / O-proj rotators.
    psA = ctx.enter_context(tc.tile_pool(name="psA", bufs=2, space="PSUM"))
    psB = ctx.enter_context(tc.tile_pool(name="psB", bufs=2, space="PSUM"))
    psD = ctx.enter_context(tc.tile_pool(name="psD", bufs=1, space="PSUM"))
    psP = ctx.enter_context(tc.tile_pool(name="psP", bufs=1, space="PSUM"))

    xqT_r = xqT[:].rearrange("(c p) s -> p c s", p=P)
    xkT_r = xkT[:].rearrange("(c p) s -> p c s", p=P)
    xvT_r = xvT[:].rearrange("(c p) s -> p c s", p=P)

    # ---- constants / weights (contiguous DMAs, ordered for fast start) ----
    wq_sb = consts.tile([P, 8, GW], BF16)
    wk_sb = consts.tile([P, 8, GW], BF16)
    wv_sb = consts.tile([P, 8, GW], BF16)
    wo_sb = consts.tile([P, 2, D_MODEL], BF16)
    bq_sb = consts.tile([P, 2], F32)
    bk_sb = consts.tile([P, 2], F32)
    bv_row = consts.tile([1, GW], F32)
    bvb = consts.tile([P, GW], F32)
    ones_f32 = consts.tile([P, 8, HPG], F32)
    warm = consts.tile([1, 1], F32)

    nc.sync.dma_start(wq_sb[:], wqP[:])
    xq0 = xb.tile([P, 8, 1024], BF16, tag="xb", name="xq0")
    nc.sync.dma_start(xq0[:], xqT_r[:, :, 0:1024])
    nc.gpsimd.dma_start(wk_sb[:], wkP[:])
    xk0 = xb.tile([P, 8, 1024], BF16, tag="xb", name="xk0")
    nc.gpsimd.dma_start(xk0[:], xkT_r[:, :, 0:1024])
    nc.gpsimd.dma_start(bq_sb[:], bq2[:])
    nc.gpsimd.dma_start(bk_sb[:], bk2[:])
    nc.gpsimd.dma_start(bv_row[:], bvr[:])
    nc.gpsimd.dma_start(wo_sb[:], woP[:])
    nc.sync.dma_start(wv_sb[:], wvP[:])

    nc.vector.memset(ones_f32[:], 1.0)
    nc.gpsimd.partition_broadcast(bvb[:], bv_row[:])
    # warm the ACT exp table during the prologue so the first real exp
    # doesn't pay the ~1.3us ACT_TABLE_LOAD
    nc.scalar.activation(out=warm[:], in_=ones_f32[0:1, 0, 0:1], func=EXPF)

    # persistent activations (QT doubles as O.T after attention)
    QTs = [persist.tile([P, S], BF16, name=f"QT{j}") for j in range(2)]
    KT4 = [[persist.tile([P, 1024], BF16, name=f"KT{j}_{hh}") for hh in range(2)]
           for j in range(2)]
    Vaugs = [persist.tile([P, 8, HPG, DK + 1], BF16, name=f"Vaug{v}")
             for v in range(2)]
    for v in range(2):
        nc.vector.tensor_scalar_add(Vaugs[v][:, :, :, DK], ones_f32[:], 0.0)

    # xk for s-half1 (consumed by K-proj filler inside head0 of q-block0)
    xk1 = xb.tile([P, 8, 1024], BF16, tag="xb", name="xk1")
    nc.gpsimd.dma_start(xk1[:], xkT_r[:, :, 1024:2048])

    # ---- pre-phase: Q proj (s-half0), K proj (s-half0), V chunks 0..7 ----
    def proj_i_major(xt, w_sb_, b_sb_, write):
        ps = [psA.tile([P, 1024], F32, tag="psA", name=f"ps_{_uid()}_{j}")
              for j in range(2)]
        for i in range(8):
            for j in range(2):
                for ns in range(2):
                    nc.tensor.matmul(
                        ps[j][:, ns * 512:(ns + 1) * 512],
                        w_sb_[:, i, j * P:(j + 1) * P],
                        xt[:, i, ns * 512:(ns + 1) * 512],
                        start=(i == 0), stop=(i == 7),
                    )
        for j in range(2):
            write(j, ps[j])

    proj_i_major(
        xq0, wq_sb, bq_sb,
        lambda j, ps: nc.vector.tensor_scalar_add(QTs[j][:, 0:1024], ps[:],
                                                  bq_sb[:, j:j + 1]))
    proj_i_major(
        xk0, wk_sb, bk_sb,
        lambda j, ps: nc.vector.tensor_scalar_add(KT4[j][0][:, :], ps[:],
                                                  bk_sb[:, j:j + 1]))

    # V chunk builder: chunk g covers s-positions [g*128, (g+1)*128)
    def v_chunk_quanta(g, prefetch=False):
        xvt = xs.tile([P, 8, P], BF16, tag="xs", name=f"xv{g}")
        if prefetch:
            nc.sync.dma_start(xvt[:], xvT_r[:, :, g * P:(g + 1) * P])
        state = {}

        def qa():
            if not prefetch:
                nc.sync.dma_start(xvt[:], xvT_r[:, :, g * P:(g + 1) * P])
            pv = psD.tile([P, 512], F32, tag="psD", name=f"pv{g}")
            state["pv"] = pv
            for i in range(4):
                nc.tensor.matmul(pv[:, 0:GW], xvt[:, i, :], wv_sb[:, i, :],
                                 start=(i == 0), stop=False)

        def qb():
            pv = state["pv"]
            for i in range(4, 8):
                nc.tensor.matmul(pv[:, 0:GW], xvt[:, i, :], wv_sb[:, i, :],
                                 start=False, stop=(i == 7))
            nc.vector.tensor_tensor(
                Vaugs[g // 8][:, g % 8, :, 0:DK],
                pv[:, 0:GW].rearrange("p (h d) -> p h d", h=HPG),
                bvb[:].rearrange("p (h d) -> p h d", h=HPG),
                ADD,
            )
        return [qa, qb]

    for g in range(8):
        for q in v_chunk_quanta(g):
            q()

    # ---- filler quanta ------------------------------------------------
    # K proj s-half1, j-major in psP: 8 quanta of 4 matmuls
    def proj_filler_quanta(xt, w_sb_, b_sb_, write):
        items = []
        for j in range(2):
            for ns in range(2):
                state = {}

                def qa(j=j, ns=ns, state=state):
                    pp = psP.tile([P, 512], F32, tag="psP",
                                  name=f"pp_{_uid()}")
                    state["pp"] = pp
                    for i in range(4):
                        nc.tensor.matmul(
                            pp[:, :], w_sb_[:, i, j * P:(j + 1) * P],
                            xt[:, i, ns * 512:(ns + 1) * 512],
                            start=(i == 0), stop=False)

                def qb(j=j, ns=ns, state=state):
                    pp = state["pp"]
                    for i in range(4, 8):
                        nc.tensor.matmul(
                            pp[:, :], w_sb_[:, i, j * P:(j + 1) * P],
                            xt[:, i, ns * 512:(ns + 1) * 512],
                            start=False, stop=(i == 7))
                    write(j, ns, pp)
                items += [qa, qb]
        return items

    fill_qb0 = deque()
    fill_qb0.extend(proj_filler_quanta(
        xk1, wk_sb, bk_sb,
        lambda j, ns, pp: nc.vector.tensor_scalar_add(
            KT4[j][1][:, ns * 512:(ns + 1) * 512], pp[:], bk_sb[:, j:j + 1])))

    # V chunks 8..15 (2 quanta each) — appended after the K items so they
    # are consumed at head0 units 8..15 (2 per unit); xv DMAs issue now so
    # the data lands before the filler quanta run
    for g in range(8, 16):
        fill_qb0.extend(v_chunk_quanta(g, prefetch=True))

    # Q proj s-half1 (consumed during head1 of q-block0); the DMA issues
    # now and lands well before head1 starts
    xq1 = xb.tile([P, 8, 1024], BF16, tag="xb", name="xq1")
    nc.gpsimd.dma_start(xq1[:], xqT_r[:, :, 1024:2048])
    fill_qb0.extend(proj_filler_quanta(
        xq1, wq_sb, bq_sb,
        lambda j, ns, pp: nc.vector.tensor_scalar_add(
            QTs[j][:, 1024 + ns * 512:1024 + (ns + 1) * 512], pp[:],
            bq_sb[:, j:j + 1])))

    # O-proj for one 128-column block sc: 2 quanta (ms halves)
    def oproj_quanta(sc):
        state = {}

        def qa():
            pso = psD.tile([P, 512], F32, tag="psD", name=f"psoA{sc}")
            for hd in range(2):
                nc.tensor.matmul(pso[:, :], QTs[hd][:, sc * P:(sc + 1) * P],
                                 wo_sb[:, hd, 0:512],
                                 start=(hd == 0), stop=(hd == 1))
            ot = outp.tile([P, 1024], F32, tag="osb", name=f"ot{sc}")
            state["ot"] = ot
            nc.vector.tensor_copy(out=ot[:, 0:512], in_=pso[:, :])

        def qb():
            pso = psP.tile([P, 512], F32, tag="psP", name=f"psoB{sc}")
            for hd in range(2):
                nc.tensor.matmul(pso[:, :], QTs[hd][:, sc * P:(sc + 1) * P],
                                 wo_sb[:, hd, 512:1024],
                                 start=(hd == 0), stop=(hd == 1))
            ot = state["ot"]
            nc.vector.tensor_copy(out=ot[:, 512:1024], in_=pso[:, :])
            nc.sync.dma_start(out[sc * P:(sc + 1) * P, :], ot[:])
        return [qa, qb]

    # ---- attention ----------------------------------------------------
    def attn_head(h, qb, fill, budget):
        jc, pr = h // 2, 64 * (h % 2)
        st = sx.tile([P, 16, 1024], BF16, tag="stexp", name=f"st{qb}{h}")
        po = [psB.tile([P, 512], F32, tag="psB", name=f"po{qb}{h}{ns}")
              for ns in range(2)]

        def attnv(k):
            for ns in range(2):
                nc.tensor.matmul(
                    po[ns][0:DK + 1, :],
                    Vaugs[k // 8][:, k % 8, h, :],
                    st[:, k, ns * 512:(ns + 1) * 512],
                    start=(k == 0), stop=(k == 15),
                )

        for k in range(16):
            pst = psA.tile([P, 1024], F32, tag="psA", name=f"pst{qb}{h}{k}")
            for ns in range(2):
                nc.tensor.matmul(
                    pst[:, ns * 512:(ns + 1) * 512],
                    KT4[jc][k // 8][pr:pr + DK, (k % 8) * P:(k % 8 + 1) * P],
                    QTs[jc][pr:pr + DK, qb * 1024 + ns * 512:
                            qb * 1024 + (ns + 1) * 512],
                    start=True, stop=True,
                )
            nc.scalar.activation(out=st[:, k, :], in_=pst[:], func=EXPF,
                                 scale=0.125)
            if k > 0:
                attnv(k - 1)
            for _ in range(budget(k)):
                if fill:
                    fill.popleft()()
        attnv(15)

        # normalize: row DK of each po half holds softmax denominators
        bc = nrm.tile([DK, 1024], F32, tag="bcast", name=f"bc{qb}{h}")
        dn = nrm.tile([1, 1024], F32, tag="denom", name=f"dn{qb}{h}")
        for ns in range(2):
            nc.vector.tensor_copy(out=dn[:, ns * 512:(ns + 1) * 512],
                                  in_=po[ns][DK:DK + 1, :])
        nc.vector.reciprocal_approx_fast(bc[0:1, :], dn[:])
        nc.gpsimd.partition_broadcast(bc[:], bc[0:1, :])
        # write O.T for this (head, q-block) into QT's now-dead region
        for ns in range(2):
            nc.vector.tensor_tensor(
                QTs[jc][pr:pr + DK,
                        qb * 1024 + ns * 512:qb * 1024 + (ns + 1) * 512],
                po[ns][0:DK, :], bc[:, ns * 512:(ns + 1) * 512], MULT)

    # q-block 0: filler = K(sb1) during h0 k0-7, V chunks 8-15 during h0
    # k8-15 (2/unit), Q(sb1) during h1 k0-7
    budgets0 = [
        lambda k: 1 if k < 8 else 2,   # h0
        lambda k: 1 if k < 8 else 0,   # h1
        lambda k: 0,                   # h2
        lambda k: 0,                   # h3
    ]
    for h in range(HPG):
        attn_head(h, 0, fill_qb0, budgets0[h])
    while fill_qb0:
        fill_qb0.popleft()()

    # q-block 1: filler = O-proj of q-block 0 (16 quanta over h0/h1)
    fill_qb1 = deque()
    for sc in range(0, 8):
        fill_qb1.extend(oproj_quanta(sc))
    budgets1 = [
        lambda k: 1 if k % 2 == 0 else 0,  # h0
        lambda k: 1 if k % 2 == 0 else 0,  # h1
        lambda k: 0,                       # h2
        lambda k: 0,                       # h3
    ]
    for h in range(HPG):
        attn_head(h, 1, fill_qb1, budgets1[h])
    while fill_qb1:
        fill_qb1.popleft()()

    # tail: O-proj of q-block 1
    for sc in range(8, 16):
        for q in oproj_quanta(sc):
            q()


_prog_cache = {}


def _build_program():
    if "nc" not in _prog_cache:
        from contextlib import ExitStack
        nc = bacc.Bacc("TRN2", target_bir_lowering=False)
        with tile.TileContext(nc) as tc:
            with ExitStack() as ctx:
                _emit(nc, tc, ctx)
        nc.compile()
        _prog_cache["nc"] = nc
    return _prog_cache["nc"]


def make_in_maps(query, key, value, Wq, bq, Wk, bk, Wv, bv, Wo, bo):
    query, key, value = (np.asarray(t, np.float32) for t in (query, key, value))
    Wq, Wk, Wv, Wo = (np.asarray(t, np.float32) for t in (Wq, Wk, Wv, Wo))
    bq, bk, bv = (np.asarray(t, np.float32) for t in (bq, bk, bv))
    xT = {b: {} for b in range(B)}
    for b in range(B):
        xT[b]["q"] = np.ascontiguousarray(query[b].T).astype(np.float16)
        xT[b]["k"] = np.ascontiguousarray(key[b].T).astype(np.float16)
        xT[b]["v"] = np.ascontiguousarray(value[b].T).astype(np.float16)

    def pack_w(WT, chunks, width):
        # [D, width] -> [128, chunks, width]; element [p,c,j] = WT[c*128+p, j]
        return np.ascontiguousarray(
            WT.reshape(chunks, 128, width).transpose(1, 0, 2)
        ).astype(np.float16)

    in_maps = []
    for c in range(N_CORES):
        b, g = divmod(c, GROUPS)
        gs = slice(g * GW, (g + 1) * GW)
        in_maps.append({
            "xqT": xT[b]["q"], "xkT": xT[b]["k"], "xvT": xT[b]["v"],
            "wqP": pack_w(Wq[gs, :].T, 8, GW),
            "wkP": pack_w(Wk[gs, :].T, 8, GW),
            "wvP": pack_w(Wv[gs, :].T, 8, GW),
            "woP": pack_w(Wo[:, gs].T, 2, D_MODEL),
            "bq2": np.ascontiguousarray(bq[gs].reshape(2, 128).T),
            "bk2": np.ascontiguousarray(bk[gs].reshape(2, 128).T),
            "bvr": np.ascontiguousarray(bv[gs].reshape(1, GW)),
        })
    return in_maps


def run_on_hw(in_maps, trace=False, **kw):
    nc = _build_program()
    return run_bass_kernel_spmd(nc, in_maps, core_ids=list(range(N_CORES)),
                                trace=trace, **kw)


def kernel(query, key, value, Wq, bq, Wk, bk, Wv, bv, Wo, bo):
    in_maps = make_in_maps(query, key, value, Wq, bq, Wk, bk, Wv, bv, Wo, bo)
    res = run_on_hw(in_maps)
    out = np.zeros((B, S, D_MODEL), np.float32)
    for c in range(N_CORES):
        out[c // GROUPS] += res.results[c]["out"]
    out += np.asarray(bo, np.float32)
    return out


if __name__ == "__main__":
    # self-check against a pure-numpy reference
    rng = np.random.default_rng(0)
    sc = 1.0 / np.sqrt(D_MODEL)
    inp = dict(
        query=rng.standard_normal((B, S, D_MODEL), np.float32),
        key=rng.standard_normal((B, S, D_MODEL), np.float32),
        value=rng.standard_normal((B, S, D_MODEL), np.float32),
        Wq=(rng.standard_normal((D_MODEL, D_MODEL)) * sc).astype(np.float32),
        bq=rng.standard_normal(D_MODEL).astype(np.float32) * 0.1,
        Wk=(rng.standard_normal((D_MODEL, D_MODEL)) * sc).astype(np.float32),
        bk=rng.standard_normal(D_MODEL).astype(np.float32) * 0.1,
        Wv=(rng.standard_normal((D_MODEL, D_MODEL)) * sc).astype(np.float32),
        bv=rng.standard_normal(D_MODEL).astype(np.float32) * 0.1,
        Wo=(rng.standard_normal((D_MODEL, D_MODEL)) * sc).astype(np.float32),
        bo=rng.standard_normal(D_MODEL).astype(np.float32) * 0.1,
    )

    def np_ref(query, key, value, Wq, bq, Wk, bk, Wv, bv, Wo, bo):
        q = query.astype(np.float64) @ Wq.T.astype(np.float64) + bq
        k = key.astype(np.float64) @ Wk.T.astype(np.float64) + bk
        v = value.astype(np.float64) @ Wv.T.astype(np.float64) + bv
        q = q.reshape(B, S, NUM_HEADS, DK).transpose(0, 2, 1, 3)
        k = k.reshape(B, S, NUM_HEADS, DK).transpose(0, 2, 1, 3)
        v = v.reshape(B, S, NUM_HEADS, DK).transpose(0, 2, 1, 3)
        sc_ = np.einsum("bhqd,bhkd->bhqk", q, k) / np.sqrt(DK)
        sc_ -= sc_.max(-1, keepdims=True)
        a = np.exp(sc_)
        a /= a.sum(-1, keepdims=True)
        o = np.einsum("bhqk,bhkd->bhqd", a, v)
        o = o.transpose(0, 2, 1, 3).reshape(B, S, D_MODEL)
        return o @ Wo.T.astype(np.float64) + bo

    exp = np_ref(**inp)
    got = kernel(**inp)
    scale = np.abs(exp).max()
    err = np.abs(got - exp)
    print(f"max abs err {err.max():.4e}  rel {err.max() / scale:.4e}  "
          f"mean rel {err.mean() / scale:.4e}")
